# revision 32
# baseline (speedup 1.0000x reference)
"""Trainium2 Bass kernel for nn_DependencyParsing (embedding_lookup).

Strategy (pure data-parallel over 8 NeuronCores, B=65536 -> 8192/core):
  - word_table cast to bf16, rows padded to 256B with a constant-1 at
    column 100 (carries the combined bias exactly through the word
    matmuls); word embeddings gathered feature-major into SBUF via SWDGE
    transpose dma_gather on 2 queues (3+ queues corrupt packets under
    concurrent HWDGE). GIDX indices per gather (1024 when the ucode
    allows, halving the ~1us fixed descriptor-gen cost per gather).
  - pos/dep lookups via one-hot matmuls in fp8e4: ONE DVE is_equal per
    chunk builds all 7 slots' one-hots [128, 7*512] fp8 from a
    host-replicated fp16 index image; slot pairs (0,1)(2,3)(4,5) run as
    DoubleRow fp8 matmuls against a host-packed projected table
    proj[slot] = [pos_table@Wp_t ; dep_table@Wd_t] (fp8, 704-stride so
    the DoubleRow weight AP step stays %16==0), slot 6 as a single fp8
    matmul.
  - h accumulates f32 in PSUM over 11 matmuls per M-tile (7 bf16 word +
    3 DR fp8 + 1 fp8). h3 = 16*h^3 via ACT Square(4*h) -> bf16 and DVE
    mul -> fp8 (the x16 scale keeps h3 out of the fp8 subnormal range;
    Wo is pre-divided by 16).
  - logits: h3 @ (Wo/16) with 2 DoubleRow fp8 + 2 single matmuls.
    Softmax is division-free: ex = Exp(logits+bo) bf16, S = ones@ex,
    then a (-1s) x Ln(S) fp32r matmul ACCUMULATES -ln(S) into the
    logits PSUM bank, and out = Exp(logits - lnS + bo) -> bf16 straight
    to DRAM. No reciprocal, no DVE epilogue.
  - The previous chunk's epilogue matmuls are interleaved between the
    current chunk's M-tiles so the PE never idles long enough for the
    HAM clock gate to re-throttle.
"""

import os

import numpy as np
import ml_dtypes

import types

import concourse.bacc as bacc
import concourse.mybir as mybir
import concourse.tile as tile
from concourse.tile import add_dep_helper
from concourse.bass_utils import run_bass_kernel_spmd


def _pin_act_tables(nc):
    """Restrict the act-table picker to the one set that covers every
    activation this kernel uses (square/exp/ln/copy), so a single
    InstLoadActFuncSet is hoisted to the top instead of 2 reloads/chunk
    (1.28us each) thrashing between the exp and ln sets. Indices into
    act_info.json are preserved (other sets are offered as empty)."""
    import bass_rust as _bass_rust
    from concourse.hw_specs import get_activation_tables

    def insert_act_table_loads(self):
        has_activation = any(
            isinstance(i, mybir.InstActivation)
            for b in self.main_func.blocks
            for i in b.instructions
        )
        if not has_activation:
            return
        keep = "natural_log_exp_and_others"
        tables = [
            (name, (s if name == keep else set()))
            for name, s in get_activation_tables(self.m.arch).items()
        ]
        _bass_rust.insert_act_table_loads(self, tables)

    nc.insert_act_table_loads = types.MethodType(insert_act_table_loads, nc)

B, T, D, H, V, NPOS, NDEP, OUT = 65536, 7, 100, 700, 32000, 50, 45, 93
NCORES = 8
B_CORE = B // NCORES
CHUNK = 512
P = 128
PS = 704  # proj slot stride (DoubleRow weight AP step must be %16)
# M-tiles over the 700 output features of h
MT = [(0, 128), (128, 128), (256, 128), (384, 128), (512, 128), (640, 60)]
# K-blocks for logits: 700 h-features in 6 blocks of 128 (last 60)
LKB = [(0, 128), (128, 128), (256, 128), (384, 128), (512, 128), (640, 60)]
dt = mybir.dt
bf16 = ml_dtypes.bfloat16
f8 = ml_dtypes.float8_e4m3
NQ = int(os.environ.get("KERNEL_NQ", "2"))
GIDX = int(os.environ.get("KERNEL_GIDX", "512"))  # indices per gather (HW cap)

_NC_CACHE = {}


def build_nc(b_core):
    n_chunks = b_core // CHUNK
    n_g = b_core // GIDX          # gathers per slot
    cpg = GIDX // CHUNK           # chunks per gather
    DR = mybir.MatmulPerfMode.DoubleRow
    nc = bacc.Bacc(None, target_bir_lowering=False, num_swdge_queues=max(NQ, 2))
    _pin_act_tables(nc)
    with tile.TileContext(nc) as tc:
        with tc.tile_pool(name="dram", bufs=1, space="DRAM") as dram:
            word_tab = dram.tile([V + 1, 128], dt.bfloat16, kind="ExternalInput",
                                 name="word_tab", uniquify=False)
            widx_d = dram.tile([P, n_g * T * (GIDX // 16)], dt.int16,
                               kind="ExternalInput", name="widx", uniquify=False)
            vidx_d = dram.tile([P, n_chunks * T * CHUNK], dt.uint8,
                               kind="ExternalInput", name="vidx", uniquify=False)
            iota_d = dram.tile([P, 1], dt.float32, kind="ExternalInput",
                               name="iota64", uniquify=False)
            ww_d = dram.tile([P, T * H], dt.bfloat16, kind="ExternalInput",
                             name="w_word", uniquify=False)
            proj_d = dram.tile([P, T * PS], dt.float8e4, kind="ExternalInput",
                               name="proj8", uniquify=False)
            wo_d = dram.tile([P, 6 * 96], dt.float8e4, kind="ExternalInput",
                             name="w_o", uniquify=False)
            bo_d = dram.tile([P, 1], dt.float32, kind="ExternalInput",
                             name="bo_pad", uniquify=False)
            negr_d = dram.tile([1, 96], dt.float32r, kind="ExternalInput",
                               name="neg_row", uniquify=False)
            out_d = dram.tile([OUT, b_core], dt.bfloat16, kind="ExternalOutput",
                              name="out", uniquify=False)

            with (
                tc.tile_pool(name="const", bufs=1) as const,
                tc.tile_pool(name="wg", bufs=6) as wg_pool,
                tc.tile_pool(name="vx", bufs=4) as vx_pool,
                tc.tile_pool(name="oh", bufs=4) as oh_pool,
                tc.tile_pool(name="sq", bufs=3) as sq_pool,
                tc.tile_pool(name="h3", bufs=3) as h3_pool,
                tc.tile_pool(name="exq", bufs=2) as ex_pool,
                tc.tile_pool(name="lnq", bufs=2) as ln_pool,
                tc.tile_pool(name="opq", bufs=2) as op_pool,
                tc.tile_pool(name="hps", bufs=1, space="PSUM") as hps_pool,
                tc.tile_pool(name="ltps", bufs=2, space="PSUM") as ltps_pool,
            ):
                preloads = []
                # widx is laid out gather-major: slices for gather-round g of
                # all 7 slots are contiguous, so round 0's indices arrive in a
                # small early DMA and the first gathers start ~10us sooner.
                GW = T * (GIDX // 16)  # widx cols per gather round
                widx_sb = const.tile([P, n_g * GW], dt.int16, name="widx_sb")
                widx_pl0 = nc.sync.dma_start(out=widx_sb[:, :GW],
                                             in_=widx_d[:, :GW])
                widx_pl1 = nc.sync.dma_start(out=widx_sb[:, GW:],
                                             in_=widx_d[:, GW:])
                ww_sb = const.tile([P, T * H], dt.bfloat16, name="ww_sb")
                preloads.append(nc.sync.dma_start(out=ww_sb[:], in_=ww_d[:]))
                proj_sb = const.tile([P, T * PS], dt.float8e4, name="proj_sb")
                preloads.append(nc.sync.dma_start(out=proj_sb[:], in_=proj_d[:]))
                wo_sb = const.tile([P, 6 * 96], dt.float8e4, name="wo_sb")
                preloads.append(nc.sync.dma_start(out=wo_sb[:], in_=wo_d[:]))
                iota_sb = const.tile([P, 1], dt.float32, name="iota_sb")
                preloads.append(nc.sync.dma_start(out=iota_sb[:], in_=iota_d[:]))
                bo_sb = const.tile([P, 1], dt.float32, name="bo_sb")
                preloads.append(nc.sync.dma_start(out=bo_sb[:], in_=bo_d[:]))
                ones_col = const.tile([P, 1], dt.bfloat16, name="ones_col")
                nc.vector.memset(ones_col[:, :], 1.0)
                neg_row = const.tile([1, 96], dt.float32r, name="neg_row_sb")
                preloads.append(nc.sync.dma_start(out=neg_row[:], in_=negr_d[:]))

                projv = proj_sb.rearrange("p (s m) -> p s m", s=T)
                wov = wo_sb.rearrange("p (s m) -> p s m", s=6)

                # Deferred epilogue pieces for the previous chunk.
                pend = {}

                def emit_logits(h3q, h3_4, h3_5):
                    lg = ltps_pool.tile([P, CHUNK], dt.float32, name="lg", tag="lt")
                    h3qv = h3q.rearrange("p (s n) -> p s n", s=4)
                    nc.tensor.matmul(lg[:96, :], wov[:, 0:2, :96], h3qv[:, 0:2, :],
                                     start=True, stop=False, perf_mode=DR)
                    nc.tensor.matmul(lg[:96, :], wov[:, 2:4, :96], h3qv[:, 2:4, :],
                                     start=False, stop=False, perf_mode=DR)
                    nc.tensor.matmul(lg[:96, :], wov[:, 4:5, :96], h3_4[:, :],
                                     start=False, stop=False)
                    nc.tensor.matmul(lg[:96, :], wov[:60, 5:6, :96], h3_5[:60, :],
                                     start=False, stop=True)
                    ex = ex_pool.tile([P, CHUNK], dt.bfloat16, name="ex")
                    nc.scalar.activation(ex[:OUT, :], lg[:OUT, :],
                                         mybir.ActivationFunctionType.Exp,
                                         bias=bo_sb[:OUT, :])
                    pend["lg"] = lg
                    pend["ex"] = ex

                def emit_sum_ln():
                    sum_ps = ltps_pool.tile([P, CHUNK], dt.float32, name="sum_ps",
                                            tag="lt")
                    nc.tensor.matmul(sum_ps[:1, :], ones_col[:OUT, :],
                                     pend["ex"][:OUT, :], start=True, stop=True)
                    lns = ln_pool.tile([1, CHUNK], dt.float32r, name="lns")
                    nc.scalar.activation(lns[:1, :], sum_ps[:1, :],
                                         mybir.ActivationFunctionType.Ln)
                    pend["lns"] = lns

                def emit_out(cc):
                    lg = pend["lg"]
                    # lg += broadcast(-ln(S)) : fp32r ones-outer-product
                    nc.tensor.matmul(lg[:96, :], neg_row[:1, :], pend["lns"][:1, :],
                                     start=False, stop=True, skip_group_check=True)
                    opt = op_pool.tile([P, CHUNK], dt.bfloat16, name="opt")
                    nc.scalar.activation(opt[:OUT, :], lg[:OUT, :],
                                         mybir.ActivationFunctionType.Exp,
                                         bias=bo_sb[:OUT, :])
                    nc.sync.dma_start(out=out_d[:, cc * CHUNK:(cc + 1) * CHUNK],
                                      in_=opt[:OUT, :])

                def make_oh(c):
                    # one-hots for all 7 slots of chunk c in ONE DVE op,
                    # issued a chunk ahead so the PE never waits on it
                    vx = vx_pool.tile([P, T * CHUNK], dt.uint8, name="vx")
                    nc.sync.dma_start(
                        out=vx[:], in_=vidx_d[:, c * T * CHUNK:(c + 1) * T * CHUNK])
                    oh = oh_pool.tile([P, T * CHUNK], dt.float8e4, name="oh")
                    nc.vector.tensor_scalar(
                        oh[:, :], vx[:, :], iota_sb[:, :], None,
                        mybir.AluOpType.is_equal)
                    return oh

                qn = 0
                prev = None
                wg_cur = None
                oh_next = make_oh(0)
                for c in range(n_chunks):
                    g_i, g_off = divmod(c, cpg)
                    if g_off == 0:
                        # ---- word gathers (feature-major), GIDX idx each ----
                        wg_cur = []
                        for t in range(T):
                            g = wg_pool.tile([P, GIDX], dt.bfloat16, name=f"wg{t}")
                            gi = nc.gpsimd.dma_gather(
                                g.rearrange("p (o n) -> p o n", o=1),
                                word_tab[:],
                                widx_sb[:, g_i * GW + t * (GIDX // 16):
                                        g_i * GW + (t + 1) * (GIDX // 16)],
                                GIDX, GIDX, 128, transpose=True, queue_num=qn % NQ,
                            )
                            if c == 0:
                                # first gathers need only their index source;
                                # SWDGE already overlaps HWDGE vidx streams at
                                # 2 queues every chunk without corruption
                                add_dep_helper(gi.ins, widx_pl0.ins)
                            elif g_i == 1:
                                add_dep_helper(gi.ins, widx_pl1.ins)
                            qn += 1
                            wg_cur.append(g)
                    wg = [g[:, g_off * CHUNK:(g_off + 1) * CHUNK] for g in wg_cur]

                    oh = oh_next
                    ohv = oh.rearrange("p (s n) -> p s n", s=T)
                    if c + 1 < n_chunks:
                        oh_next = make_oh(c + 1)

                    # ---- h = x @ W ; h3 = 16*h^3 as fp8 ----
                    # PE work is phase-grouped by operand mode (42 bf16 word
                    # matmuls, then all fp8 DR/single matmuls incl. the prev
                    # chunk's logits, then the bf16/f32r softmax matmuls):
                    # each bf16<->fp8 weight-path mode switch costs ~130ns,
                    # so per-M-tile interleaving wastes ~2us/chunk.
                    hps = [hps_pool.tile([P, CHUNK], dt.float32, name=f"hps{mi}")
                           for mi in range(6)]
                    # t outer / mi inner: each gather feeds 6 matmuls before
                    # the next gather is needed, rate-matching the ~1.8us
                    # SWDGE arrival spacing during pipeline fill
                    for t in range(T):
                        for mi, (m0, msz) in enumerate(MT):
                            nc.tensor.matmul(
                                hps[mi][:msz, :],
                                ww_sb[:, t * H + m0: t * H + m0 + msz],
                                wg[t],
                                start=(t == 0), stop=False,
                            )
                    if prev is not None:
                        emit_logits(*prev)
                    h3q = h3_pool.tile([P, 4 * CHUNK], dt.float8e4, name="h3q")
                    h3qv = h3q.rearrange("p (s n) -> p s n", s=4)
                    h3_4 = h3_pool.tile([P, CHUNK], dt.float8e4, name="h3_4")
                    h3_5 = h3_pool.tile([P, CHUNK], dt.float8e4, name="h3_5")
                    for mi, (m0, msz) in enumerate(MT):
                        hp = hps[mi]
                        for j in range(3):
                            nc.tensor.matmul(
                                hp[:msz, :],
                                projv[:, 2 * j:2 * j + 2, m0:m0 + msz],
                                ohv[:, 2 * j:2 * j + 2, :],
                                start=False, stop=False, perf_mode=DR,
                            )
                        nc.tensor.matmul(
                            hp[:msz, :], projv[:, 6:7, m0:m0 + msz], ohv[:, 6, :],
                            start=False, stop=True,
                        )
                        sq = sq_pool.tile([P, CHUNK], dt.bfloat16, name="sq")
                        nc.scalar.activation(sq[:msz, :], hp[:msz, :],
                                             mybir.ActivationFunctionType.Square,
                                             scale=4.0)
                        if mi < 4:
                            h3t = h3qv[:msz, mi, :]
                        elif mi == 4:
                            h3t = h3_4[:msz, :]
                        else:
                            h3t = h3_5[:msz, :]
                        nc.vector.tensor_mul(h3t, sq[:msz, :], hp[:msz, :])
                    if prev is not None:
                        emit_sum_ln()
                        emit_out(c - 1)
                    prev = (h3q, h3_4, h3_5)

                # tail epilogue for the last chunk
                emit_logits(*prev)
                emit_sum_ln()
                emit_out(n_chunks - 1)
    nc.compile()
    return nc


def _wrap_idx(idx_tc):
    """[GIDX] -> [128, GIDX//16] wrapped (i -> [i%16, i//16]) + replicated x8."""
    n = idx_tc.shape[0]
    w = idx_tc.reshape(n // 16, 16).T  # [16, n/16]
    return np.tile(w, (8, 1))


def prep_inputs(word_idx, pos_idx, dep_idx, word_table, pos_table, dep_table,
                Ww, bw, Wp, bp, Wd, bd, Wo, bo, b_core):
    """Returns (shared_map, per_core_fn). Host work is layout + tiny matmuls."""
    n_chunks = b_core // CHUNK
    n_g = b_core // GIDX

    bias_all = (np.asarray(bw, np.float32) + np.asarray(bp, np.float32)
                + np.asarray(bd, np.float32))

    wt = np.zeros((V + 1, 128), dtype=bf16)
    wt[:V, :D] = np.asarray(word_table, np.float32).astype(bf16)
    wt[:, D] = bf16(1.0)  # constant-1 column carries the bias via slot 0

    def pack_w(Wmat):
        arr = np.zeros((T, P, H), dtype=bf16)
        Wmat = np.asarray(Wmat, np.float32)
        for t in range(T):
            arr[t, :D, :] = Wmat[D * t:D * (t + 1), :].astype(bf16)
        return arr

    ww = pack_w(Ww)
    ww[0, D, :] = bias_all.astype(bf16)  # bias row rides word slot 0's 1-col

    # proj8[p, t, :]: p<64 -> pos_table@Wp_t rows, p>=64 -> dep_table@Wd_t
    Wp32 = np.asarray(Wp, np.float32)
    Wd32 = np.asarray(Wd, np.float32)
    pt = np.asarray(pos_table, np.float32)
    dtab = np.asarray(dep_table, np.float32)
    proj8 = np.zeros((P, T, PS), dtype=f8)
    for t in range(T):
        proj8[:NPOS, t, :H] = (pt @ Wp32[D * t:D * (t + 1), :]).astype(f8)
        proj8[64:64 + NDEP, t, :H] = (dtab @ Wd32[D * t:D * (t + 1), :]).astype(f8)

    wo8 = np.zeros((6, P, 96), dtype=f8)
    Wo16 = np.asarray(Wo, np.float32) / 16.0  # h3 carries x16
    for j, (k0, ksz) in enumerate(LKB):
        wo8[j, :ksz, :OUT] = Wo16[k0:k0 + ksz, :].astype(f8)

    bo_pad = np.zeros((P, 1), dtype=np.float32)
    bo_pad[:OUT, 0] = np.asarray(bo, np.float32)

    iota64 = (np.arange(P) % 64).astype(np.float32).reshape(P, 1)

    shared = {
        "word_tab": wt,
        "iota64": iota64,
        "w_word": np.ascontiguousarray(ww.transpose(1, 0, 2)).reshape(P, T * H),
        "proj8": proj8.reshape(P, T * PS),
        "w_o": np.ascontiguousarray(wo8.transpose(1, 0, 2)).reshape(P, 6 * 96),
        "bo_pad": bo_pad,
        "neg_row": np.full((1, 96), -1.0, np.float32),
    }

    wi = np.asarray(word_idx, np.int64).copy()
    wi[wi < 0] = V
    wi = wi.astype(np.int16)
    pi16 = np.asarray(pos_idx, np.int32).astype(np.uint8)
    di16 = np.asarray(dep_idx, np.int32).astype(np.uint8)

    def core_map(core):
        s = slice(core * b_core, (core + 1) * b_core)
        wic = wi[s]
        widx = np.zeros((P, n_g, T, GIDX // 16), dtype=np.int16)
        for t in range(T):
            for g in range(n_g):
                widx[:, g, t, :] = _wrap_idx(wic[g * GIDX:(g + 1) * GIDX, t])

        # vidx[p, c, t, i]: p<64 -> pos_idx, p>=64 -> dep_idx
        pc = pi16[s].reshape(n_chunks, CHUNK, T).transpose(0, 2, 1)
        dc = di16[s].reshape(n_chunks, CHUNK, T).transpose(0, 2, 1)
        vidx = np.empty((P, n_chunks, T, CHUNK), dtype=np.uint8)
        vidx[:64] = pc[None, :, :, :]
        vidx[64:] = dc[None, :, :, :]

        m = dict(shared)
        m["widx"] = widx.reshape(P, n_g * T * (GIDX // 16))
        m["vidx"] = np.ascontiguousarray(vidx).reshape(P, n_chunks * T * CHUNK)
        return m

    return shared, core_map


def kernel(**inputs):
    b_core = B_CORE
    if b_core not in _NC_CACHE:
        _NC_CACHE[b_core] = build_nc(b_core)
    nc = _NC_CACHE[b_core]

    _, core_map = prep_inputs(b_core=b_core, **inputs)
    in_maps = [core_map(i) for i in range(NCORES)]
    res = run_bass_kernel_spmd(nc, in_maps, core_ids=list(range(NCORES)))
    out = np.concatenate([r["out"] for r in res.results], axis=1)  # [93, B] bf16
    return np.ascontiguousarray(out.T).astype(np.float32)


# revision 39
# speedup vs baseline: 1.1963x; 1.1963x over previous
"""Trainium2 Bass kernel for nn_DependencyParsing (embedding_lookup).

Strategy (pure data-parallel over 8 NeuronCores, B=65536 -> 8192/core):
  - word_table cast to bf16, rows padded to 256B with a constant-1 at
    column 100 (carries the combined bias exactly through the word
    matmuls); word embeddings gathered feature-major into SBUF via SWDGE
    transpose dma_gather on 2 queues (3+ queues corrupt packets under
    concurrent HWDGE). GIDX indices per gather (1024 when the ucode
    allows, halving the ~1us fixed descriptor-gen cost per gather).
  - pos/dep lookups via one-hot matmuls in fp8e4: ONE DVE is_equal per
    chunk builds all 7 slots' one-hots [128, 7*512] fp8 from a
    host-replicated fp16 index image; slot pairs (0,1)(2,3)(4,5) run as
    DoubleRow fp8 matmuls against a host-packed projected table
    proj[slot] = [pos_table@Wp_t ; dep_table@Wd_t] (fp8, 704-stride so
    the DoubleRow weight AP step stays %16==0), slot 6 as a single fp8
    matmul.
  - h accumulates f32 in PSUM over 11 matmuls per M-tile (7 bf16 word +
    3 DR fp8 + 1 fp8). h3 = 16*h^3 via ACT Square(4*h) -> bf16 and DVE
    mul -> fp8 (the x16 scale keeps h3 out of the fp8 subnormal range;
    Wo is pre-divided by 16).
  - logits: h3 @ (Wo/16) with 2 DoubleRow fp8 + 2 single matmuls.
    Softmax is division-free: ex = Exp(logits+bo) bf16, S = ones@ex,
    then a (-1s) x Ln(S) fp32r matmul ACCUMULATES -ln(S) into the
    logits PSUM bank, and out = Exp(logits - lnS + bo) -> bf16 straight
    to DRAM. No reciprocal, no DVE epilogue.
  - The previous chunk's epilogue matmuls are interleaved between the
    current chunk's M-tiles so the PE never idles long enough for the
    HAM clock gate to re-throttle.
"""

import os

import numpy as np
import ml_dtypes

import types

import concourse.bacc as bacc
import concourse.mybir as mybir
import concourse.tile as tile
from concourse.tile import add_dep_helper
from concourse.bass_utils import run_bass_kernel_spmd


def _pin_act_tables(nc):
    """Restrict the act-table picker to the one set that covers every
    activation this kernel uses (square/exp/ln/copy), so a single
    InstLoadActFuncSet is hoisted to the top instead of 2 reloads/chunk
    (1.28us each) thrashing between the exp and ln sets. Indices into
    act_info.json are preserved (other sets are offered as empty)."""
    import bass_rust as _bass_rust
    from concourse.hw_specs import get_activation_tables

    def insert_act_table_loads(self):
        has_activation = any(
            isinstance(i, mybir.InstActivation)
            for b in self.main_func.blocks
            for i in b.instructions
        )
        if not has_activation:
            return
        keep = "natural_log_exp_and_others"
        tables = [
            (name, (s if name == keep else set()))
            for name, s in get_activation_tables(self.m.arch).items()
        ]
        _bass_rust.insert_act_table_loads(self, tables)

    nc.insert_act_table_loads = types.MethodType(insert_act_table_loads, nc)

B, T, D, H, V, NPOS, NDEP, OUT = 65536, 7, 100, 700, 32000, 50, 45, 93
NCORES = 8
B_CORE = B // NCORES
CHUNK = 512
P = 128
PS = 704  # proj slot stride (DoubleRow weight AP step must be %16)
OHS = 6   # one-hot slot-tiles: slot 6 folds into slots 0-5's free partitions
# M-tiles over the 700 output features of h
MT = [(0, 128), (128, 128), (256, 128), (384, 128), (512, 128), (640, 60)]
# K-blocks for logits: 700 h-features in 6 blocks of 128 (last 60)
LKB = [(0, 128), (128, 128), (256, 128), (384, 128), (512, 128), (640, 60)]
dt = mybir.dt
bf16 = ml_dtypes.bfloat16
f8 = ml_dtypes.float8_e4m3
NQ = int(os.environ.get("KERNEL_NQ", "2"))
GIDX = int(os.environ.get("KERNEL_GIDX", "512"))  # indices per gather (HW cap)

_NC_CACHE = {}


def build_nc(b_core):
    n_chunks = b_core // CHUNK
    n_g = b_core // GIDX          # gathers per slot
    cpg = GIDX // CHUNK           # chunks per gather
    DR = mybir.MatmulPerfMode.DoubleRow
    nc = bacc.Bacc(None, target_bir_lowering=False, num_swdge_queues=max(NQ, 2))
    _pin_act_tables(nc)
    with tile.TileContext(nc) as tc:
        with tc.tile_pool(name="dram", bufs=1, space="DRAM") as dram:
            word_tab = dram.tile([V + 1, 128], dt.bfloat16, kind="ExternalInput",
                                 name="word_tab", uniquify=False)
            widx_d = dram.tile([P, n_g * T * (GIDX // 16)], dt.int16,
                               kind="ExternalInput", name="widx", uniquify=False)
            vidx_d = dram.tile([P, n_chunks * OHS * CHUNK], dt.uint8,
                               kind="ExternalInput", name="vidx", uniquify=False)
            iota_d = dram.tile([P, 1], dt.float32, kind="ExternalInput",
                               name="iota64", uniquify=False)
            ww_d = dram.tile([P, T * H], dt.bfloat16, kind="ExternalInput",
                             name="w_word", uniquify=False)
            proj_d = dram.tile([P, OHS * PS], dt.float8e4, kind="ExternalInput",
                               name="proj8", uniquify=False)
            wo_d = dram.tile([P, 6 * 96], dt.float8e4, kind="ExternalInput",
                             name="w_o", uniquify=False)
            bo_d = dram.tile([P, 1], dt.float32, kind="ExternalInput",
                             name="bo_pad", uniquify=False)
            negr_d = dram.tile([1, 96], dt.float32r, kind="ExternalInput",
                               name="neg_row", uniquify=False)
            out_d = dram.tile([OUT, b_core], dt.bfloat16, kind="ExternalOutput",
                              name="out", uniquify=False)

            with (
                tc.tile_pool(name="const", bufs=1) as const,
                tc.tile_pool(name="wg", bufs=6) as wg_pool,
                tc.tile_pool(name="vx", bufs=4) as vx_pool,
                tc.tile_pool(name="oh", bufs=4) as oh_pool,
                tc.tile_pool(name="sq", bufs=3) as sq_pool,
                tc.tile_pool(name="h3", bufs=3) as h3_pool,
                tc.tile_pool(name="exq", bufs=2) as ex_pool,
                tc.tile_pool(name="lnq", bufs=2) as ln_pool,
                tc.tile_pool(name="opq", bufs=2) as op_pool,
                tc.tile_pool(name="hps", bufs=1, space="PSUM") as hps_pool,
                tc.tile_pool(name="ltps", bufs=2, space="PSUM") as ltps_pool,
            ):
                preloads = []
                # widx is laid out gather-major: slices for gather-round g of
                # all 7 slots are contiguous, so round 0's indices arrive in a
                # small early DMA and the first gathers start ~10us sooner.
                GW = T * (GIDX // 16)  # widx cols per gather round
                widx_sb = const.tile([P, n_g * GW], dt.int16, name="widx_sb")
                widx_pl0 = nc.sync.dma_start(out=widx_sb[:, :GW],
                                             in_=widx_d[:, :GW])
                widx_pl1 = nc.sync.dma_start(out=widx_sb[:, GW:],
                                             in_=widx_d[:, GW:])
                ww_sb = const.tile([P, T * H], dt.bfloat16, name="ww_sb")
                preloads.append(nc.sync.dma_start(out=ww_sb[:], in_=ww_d[:]))
                proj_sb = const.tile([P, OHS * PS], dt.float8e4, name="proj_sb")
                preloads.append(nc.sync.dma_start(out=proj_sb[:], in_=proj_d[:]))
                wo_sb = const.tile([P, 6 * 96], dt.float8e4, name="wo_sb")
                preloads.append(nc.sync.dma_start(out=wo_sb[:], in_=wo_d[:]))
                iota_sb = const.tile([P, 1], dt.float32, name="iota_sb")
                preloads.append(nc.sync.dma_start(out=iota_sb[:], in_=iota_d[:]))
                bo_sb = const.tile([P, 1], dt.float32, name="bo_sb")
                preloads.append(nc.sync.dma_start(out=bo_sb[:], in_=bo_d[:]))
                ones_col = const.tile([P, 1], dt.bfloat16, name="ones_col")
                nc.vector.memset(ones_col[:, :], 1.0)
                neg_row = const.tile([1, 96], dt.float32r, name="neg_row_sb")
                preloads.append(nc.sync.dma_start(out=neg_row[:], in_=negr_d[:]))

                projv = proj_sb.rearrange("p (s m) -> p s m", s=OHS)
                wov = wo_sb.rearrange("p (s m) -> p s m", s=6)

                # Deferred epilogue pieces for the previous chunk.
                pend = {}

                def emit_logits(h3q, h3_4, h3_5):
                    lg = ltps_pool.tile([P, CHUNK], dt.float32, name="lg", tag="lt")
                    h3qv = h3q.rearrange("p (s n) -> p s n", s=4)
                    nc.tensor.matmul(lg[:96, :], wov[:, 0:2, :96], h3qv[:, 0:2, :],
                                     start=True, stop=False, perf_mode=DR)
                    nc.tensor.matmul(lg[:96, :], wov[:, 2:4, :96], h3qv[:, 2:4, :],
                                     start=False, stop=False, perf_mode=DR)
                    nc.tensor.matmul(lg[:96, :], wov[:, 4:5, :96], h3_4[:, :],
                                     start=False, stop=False)
                    nc.tensor.matmul(lg[:96, :], wov[:60, 5:6, :96], h3_5[:60, :],
                                     start=False, stop=True)
                    ex = ex_pool.tile([P, CHUNK], dt.bfloat16, name="ex")
                    nc.scalar.activation(ex[:OUT, :], lg[:OUT, :],
                                         mybir.ActivationFunctionType.Exp,
                                         bias=bo_sb[:OUT, :])
                    pend["lg"] = lg
                    pend["ex"] = ex

                def emit_sum_ln():
                    sum_ps = ltps_pool.tile([P, CHUNK], dt.float32, name="sum_ps",
                                            tag="lt")
                    nc.tensor.matmul(sum_ps[:1, :], ones_col[:OUT, :],
                                     pend["ex"][:OUT, :], start=True, stop=True)
                    lns = ln_pool.tile([1, CHUNK], dt.float32r, name="lns")
                    nc.scalar.activation(lns[:1, :], sum_ps[:1, :],
                                         mybir.ActivationFunctionType.Ln)
                    pend["lns"] = lns

                def emit_out(cc):
                    lg = pend["lg"]
                    # lg += broadcast(-ln(S)) : fp32r ones-outer-product
                    nc.tensor.matmul(lg[:96, :], neg_row[:1, :], pend["lns"][:1, :],
                                     start=False, stop=True, skip_group_check=True)
                    opt = op_pool.tile([P, CHUNK], dt.bfloat16, name="opt")
                    nc.scalar.activation(opt[:OUT, :], lg[:OUT, :],
                                         mybir.ActivationFunctionType.Exp,
                                         bias=bo_sb[:OUT, :])
                    nc.sync.dma_start(out=out_d[:, cc * CHUNK:(cc + 1) * CHUNK],
                                      in_=opt[:OUT, :])

                def make_oh(c):
                    # one-hots for all 7 slots of chunk c in ONE DVE op,
                    # issued a chunk ahead so the PE never waits on it
                    vx = vx_pool.tile([P, OHS * CHUNK], dt.uint8, name="vx")
                    nc.sync.dma_start(
                        out=vx[:],
                        in_=vidx_d[:, c * OHS * CHUNK:(c + 1) * OHS * CHUNK])
                    oh = oh_pool.tile([P, OHS * CHUNK], dt.float8e4, name="oh")
                    nc.vector.tensor_scalar(
                        oh[:, :], vx[:, :], iota_sb[:, :], None,
                        mybir.AluOpType.is_equal)
                    return oh

                qn = 0
                prev = None
                wg_cur = None
                oh_next = make_oh(0)
                for c in range(n_chunks):
                    g_i, g_off = divmod(c, cpg)
                    if g_off == 0:
                        # ---- word gathers (feature-major), GIDX idx each ----
                        wg_cur = []
                        for t in range(T):
                            g = wg_pool.tile([P, GIDX], dt.bfloat16, name=f"wg{t}")
                            gi = nc.gpsimd.dma_gather(
                                g.rearrange("p (o n) -> p o n", o=1),
                                word_tab[:],
                                widx_sb[:, g_i * GW + t * (GIDX // 16):
                                        g_i * GW + (t + 1) * (GIDX // 16)],
                                GIDX, GIDX, 128, transpose=True, queue_num=qn % NQ,
                            )
                            if c == 0:
                                # first gathers need only their index source;
                                # SWDGE already overlaps HWDGE vidx streams at
                                # 2 queues every chunk without corruption
                                add_dep_helper(gi.ins, widx_pl0.ins)
                            elif g_i == 1:
                                add_dep_helper(gi.ins, widx_pl1.ins)
                            qn += 1
                            wg_cur.append(g)
                    wg = [g[:, g_off * CHUNK:(g_off + 1) * CHUNK] for g in wg_cur]

                    oh = oh_next
                    ohv = oh.rearrange("p (s n) -> p s n", s=OHS)
                    if c + 1 < n_chunks:
                        oh_next = make_oh(c + 1)

                    # ---- h = x @ W ; h3 = 16*h^3 as fp8 ----
                    # PE work is phase-grouped by operand mode (42 bf16 word
                    # matmuls, then all fp8 DR/single matmuls incl. the prev
                    # chunk's logits, then the bf16/f32r softmax matmuls):
                    # each bf16<->fp8 weight-path mode switch costs ~130ns,
                    # so per-M-tile interleaving wastes ~2us/chunk.
                    hps = []
                    for mi, (m0, msz) in enumerate(MT):
                        hp = hps_pool.tile([P, CHUNK], dt.float32, name=f"hps{mi}")
                        hps.append(hp)
                        for t in range(T):
                            nc.tensor.matmul(
                                hp[:msz, :],
                                ww_sb[:, t * H + m0: t * H + m0 + msz],
                                wg[t],
                                start=(t == 0), stop=False,
                            )
                    if prev is not None:
                        emit_logits(*prev)
                    h3q = h3_pool.tile([P, 4 * CHUNK], dt.float8e4, name="h3q")
                    h3qv = h3q.rearrange("p (s n) -> p s n", s=4)
                    h3_4 = h3_pool.tile([P, CHUNK], dt.float8e4, name="h3_4")
                    h3_5 = h3_pool.tile([P, CHUNK], dt.float8e4, name="h3_5")
                    for mi, (m0, msz) in enumerate(MT):
                        hp = hps[mi]
                        for j in range(3):
                            nc.tensor.matmul(
                                hp[:msz, :],
                                projv[:, 2 * j:2 * j + 2, m0:m0 + msz],
                                ohv[:, 2 * j:2 * j + 2, :],
                                start=False, stop=(j == 2), perf_mode=DR,
                            )
                        sq = sq_pool.tile([P, CHUNK], dt.bfloat16, name="sq")
                        nc.scalar.activation(sq[:msz, :], hp[:msz, :],
                                             mybir.ActivationFunctionType.Square,
                                             scale=4.0)
                        if mi < 4:
                            h3t = h3qv[:msz, mi, :]
                        elif mi == 4:
                            h3t = h3_4[:msz, :]
                        else:
                            h3t = h3_5[:msz, :]
                        nc.vector.tensor_mul(h3t, sq[:msz, :], hp[:msz, :])
                    if prev is not None:
                        emit_sum_ln()
                        emit_out(c - 1)
                    prev = (h3q, h3_4, h3_5)

                # tail epilogue for the last chunk
                emit_logits(*prev)
                emit_sum_ln()
                emit_out(n_chunks - 1)
    nc.compile()
    return nc


def _wrap_idx(idx_tc):
    """[GIDX] -> [128, GIDX//16] wrapped (i -> [i%16, i//16]) + replicated x8."""
    n = idx_tc.shape[0]
    w = idx_tc.reshape(n // 16, 16).T  # [16, n/16]
    return np.tile(w, (8, 1))


def prep_inputs(word_idx, pos_idx, dep_idx, word_table, pos_table, dep_table,
                Ww, bw, Wp, bp, Wd, bd, Wo, bo, b_core):
    """Returns (shared_map, per_core_fn). Host work is layout + tiny matmuls."""
    n_chunks = b_core // CHUNK
    n_g = b_core // GIDX

    bias_all = (np.asarray(bw, np.float32) + np.asarray(bp, np.float32)
                + np.asarray(bd, np.float32))

    wt = np.zeros((V + 1, 128), dtype=bf16)
    wt[:V, :D] = np.asarray(word_table, np.float32).astype(bf16)
    wt[:, D] = bf16(1.0)  # constant-1 column carries the bias via slot 0

    def pack_w(Wmat):
        arr = np.zeros((T, P, H), dtype=bf16)
        Wmat = np.asarray(Wmat, np.float32)
        for t in range(T):
            arr[t, :D, :] = Wmat[D * t:D * (t + 1), :].astype(bf16)
        return arr

    ww = pack_w(Ww)
    ww[0, D, :] = bias_all.astype(bf16)  # bias row rides word slot 0's 1-col

    # proj8[p, s, :]: p<50 pos slot s; p in 50..63 pos slot-6 fragment
    # (classes 14s..14s+13); p in 64..108 dep slot s; p in 109..127 dep
    # slot-6 fragment (classes 19s..19s+18). Slot 6 one-hots fold into the
    # free partitions of slots 0..5, making those columns 4-hot.
    Wp32 = np.asarray(Wp, np.float32)
    Wd32 = np.asarray(Wd, np.float32)
    pt = np.asarray(pos_table, np.float32)
    dtab = np.asarray(dep_table, np.float32)
    pproj = [pt @ Wp32[D * t:D * (t + 1), :] for t in range(T)]   # [50,700] x7
    dproj = [dtab @ Wd32[D * t:D * (t + 1), :] for t in range(T)]  # [45,700] x7
    proj8 = np.zeros((P, OHS, PS), dtype=f8)
    for s in range(OHS):
        proj8[:NPOS, s, :H] = pproj[s].astype(f8)
        lo = 14 * s
        if lo < NPOS:
            n = min(14, NPOS - lo)
            proj8[50:50 + n, s, :H] = pproj[6][lo:lo + n].astype(f8)
        proj8[64:64 + NDEP, s, :H] = dproj[s].astype(f8)
        lo = 19 * s
        if lo < NDEP:
            n = min(19, NDEP - lo)
            proj8[109:109 + n, s, :H] = dproj[6][lo:lo + n].astype(f8)

    wo8 = np.zeros((6, P, 96), dtype=f8)
    Wo16 = np.asarray(Wo, np.float32) / 16.0  # h3 carries x16
    for j, (k0, ksz) in enumerate(LKB):
        wo8[j, :ksz, :OUT] = Wo16[k0:k0 + ksz, :].astype(f8)

    bo_pad = np.zeros((P, 1), dtype=np.float32)
    bo_pad[:OUT, 0] = np.asarray(bo, np.float32)

    iota64 = (np.arange(P) % 64).astype(np.float32).reshape(P, 1)

    shared = {
        "word_tab": wt,
        "iota64": iota64,
        "w_word": np.ascontiguousarray(ww.transpose(1, 0, 2)).reshape(P, T * H),
        "proj8": proj8.reshape(P, OHS * PS),
        "w_o": np.ascontiguousarray(wo8.transpose(1, 0, 2)).reshape(P, 6 * 96),
        "bo_pad": bo_pad,
        "neg_row": np.full((1, 96), -1.0, np.float32),
    }

    wi = np.asarray(word_idx, np.int64).copy()
    wi[wi < 0] = V
    wi = wi.astype(np.int16)
    pi32 = np.asarray(pos_idx, np.int32)
    di32 = np.asarray(dep_idx, np.int32)

    def core_map(core):
        s = slice(core * b_core, (core + 1) * b_core)
        wic = wi[s]
        widx = np.zeros((P, n_g, T, GIDX // 16), dtype=np.int16)
        for t in range(T):
            for g in range(n_g):
                widx[:, g, t, :] = _wrap_idx(wic[g * GIDX:(g + 1) * GIDX, t])

        # vidx[p, c, s, i]: p<50 pos_s; 50..63 pos slot-6 shifted; 64..108
        # dep_s; 109..127 dep slot-6 shifted. iota[p] = p%64 throughout, so
        # the shifted values 50+pos6-14s / 45+dep6-19s hit exactly the
        # fragment partitions (collision-free mod 256 for every s).
        pc = pi32[s].reshape(n_chunks, CHUNK, T).transpose(0, 2, 1)
        dc = di32[s].reshape(n_chunks, CHUNK, T).transpose(0, 2, 1)
        sh = np.arange(OHS, dtype=np.int32)[None, :, None]
        p6 = ((50 + pc[:, 6:7, :] - 14 * sh) % 256).astype(np.uint8)
        d6 = ((45 + dc[:, 6:7, :] - 19 * sh) % 256).astype(np.uint8)
        vidx = np.empty((P, n_chunks, OHS, CHUNK), dtype=np.uint8)
        vidx[:50] = pc[None, :, :OHS, :].astype(np.uint8)
        vidx[50:64] = p6[None, :, :, :]
        vidx[64:109] = dc[None, :, :OHS, :].astype(np.uint8)
        vidx[109:] = d6[None, :, :, :]

        m = dict(shared)
        m["widx"] = widx.reshape(P, n_g * T * (GIDX // 16))
        m["vidx"] = np.ascontiguousarray(vidx).reshape(P, n_chunks * OHS * CHUNK)
        return m

    return shared, core_map


def kernel(**inputs):
    b_core = B_CORE
    if b_core not in _NC_CACHE:
        _NC_CACHE[b_core] = build_nc(b_core)
    nc = _NC_CACHE[b_core]

    _, core_map = prep_inputs(b_core=b_core, **inputs)
    in_maps = [core_map(i) for i in range(NCORES)]
    res = run_bass_kernel_spmd(nc, in_maps, core_ids=list(range(NCORES)))
    out = np.concatenate([r["out"] for r in res.results], axis=1)  # [93, B] bf16
    return np.ascontiguousarray(out.T).astype(np.float32)
